# revision 1
# baseline (speedup 1.0000x reference)
import numpy as np

N=4096; C=1024; INTER=128; R=128; RR=R*R; GC=256; NCORES=8; NB=N//NCORES
PW=130; HR=R//NCORES           # 16 output h-rows per core
WINR=HR+2                      # 18 padded rows in window
WIN=WINR*PW                    # 2340
QT=(WIN+127)//128              # 19 k-tiles for q
QPAD=QT*128                    # 2432
PWIN=2694                      # window read span
PGLOB=17280                    # padded p buffer (guard 131 + 16900 + tail)
AGS=NB*(INTER+1)+HR*R          # 512*129+2048 = 68096
ARS=9*GC+C                     # 2304+1024 = 3328
KT=C//128                      # 8

_cache = {}

def _fold(p):
    f32=np.float32
    out={}
    mcw1=p['m_cw'][:INTER]; mcw2=p['m_cw'][INTER:]
    xv=np.zeros((C,6),f32); sc=np.zeros((1,8),f32)
    xv[:,0]=p['m_tw'].T@mcw1; sc[0,0]=p['m_tb']@mcw1            # a
    for j in range(3):
        c1=p['pr_cw'][j,:INTER]; c2=p['pr_cw'][j,INTER:]
        xv[:,1+j]=p['pr_tw'][j].T@c1
        sc[0,1+j]=p['pr_tb'][j]@c1+p['pr_pb'][j]@c2
    xv[:,4]=p['ba_tw'].T@p['ba_cw'][:INTER]
    xv[:,5]=p['m_pw'].T@mcw2; sc[0,5]=p['m_pb']@mcw2            # b
    sc[0,4]=p['ba_tb']@p['ba_cw'][:INTER]+p['ba_pb']@p['ba_cw'][INTER:]
    out['xvecs']=xv; out['sconst']=sc
    vps=np.stack([p['pr_pw'][j].T@p['pr_cw'][j,INTER:] for j in range(3)],1)
    out['vps']=vps.astype(f32)                                   # [C,3]
    out['vpm']=(p['ba_pw'].T@p['ba_cw'][INTER:]/ (2*N)).astype(f32)[:,None]  # [C,1]
    out['m_gwT']=p['m_gw'].T.copy()                              # [C,128]
    out['pr_gwT']=np.stack([p['pr_gw'][j].T for j in range(3)])  # [3,C,128]
    bg=float(p['ba_g'][0])
    out['ba_gwT']=(bg*p['ba_gw'].T/(2*N)).copy()                 # [C,128]
    sg=float(p['sp_g'][0])
    # sp_gwT: [(mh*3+mw)*GC+ic, oc] with (kh,kw)=(2-mh,2-mw), scaled by sp_g
    g=np.transpose(p['sp_gw'],(2,3,1,0))[::-1,::-1]              # [kh',kw',ic,oc] reversed
    out['sp_gwT']=np.ascontiguousarray(sg*g.reshape(9*GC,INTER))
    # w_effT [2,128,9]: w_eff[ic,kh,kw]=sum_c spcw2[c]*sp_pw[c,ic,kh,kw]
    we=np.einsum('c,cikl->ikl',p['sp_cw'][INTER:],p['sp_pw'])    # [GC,3,3]
    out['w_effT']=we.reshape(2,128,9).astype(f32)
    # biases128 [128,6]: m_gb, pr_gb0..2, ba_g*ba_gb(gm bias), sp_g*sp_gb(v bias)
    b6=np.zeros((INTER,6),f32)
    b6[:,0]=p['m_gb']; b6[:,1:4]=p['pr_gb'].T; b6[:,4]=bg*p['ba_gb']; b6[:,5]=sg*p['sp_gb']
    out['bias128']=b6
    gf=np.zeros((1,4*INTER),f32)
    for j in range(3): gf[0,j*INTER:(j+1)*INTER]=p['pr_g'][j]
    gf[0,3*INTER:]=1.0
    out['gfill']=gf
    out['mgb_row']=p['m_gb'][None,:].astype(f32)                 # [1,128] K=1 bias trick
    return out

def _shard(p):
    f32=np.float32
    gpadded=np.pad(p['global_feature'][0],((0,0),(1,1),(1,1)))   # [GC,130,130]
    ins=[]
    for k in range(NCORES):
        d={}
        rs=slice(k*NB,(k+1)*NB)
        d['xT']=np.ascontiguousarray(p['origin_feature'][rs].T)
        yt=np.stack([np.ascontiguousarray(t[rs].T) for t in
                     (p['local_feature'],p['bef_l'],p['aft_l'])])
        d['yT']=yt                                               # [3,C,NB]
        d['bafT']=np.ascontiguousarray(np.concatenate(
            [p['bef'][rs],p['aft'][rs]],0).T)                    # [C,2NB]
        gw=gpadded[:,k*HR:k*HR+WINR,:]                           # [GC,18,130]
        d['gpad']=np.ascontiguousarray(gw.reshape(2,128,WINR*PW)
                    .transpose(1,0,2).reshape(128,2*WINR*PW))
        gt=np.zeros((QPAD,GC),f32)
        gt[:WIN]=gw.reshape(GC,WIN).T
        d['gpadT']=gt.reshape(QT,128,GC)
        osel=np.zeros((NCORES,1),f32); osel[k,0]=1.0
        d['osel']=osel
        ins.append(d)
    return ins

def kernel(**inputs):
    import ml_dtypes  # noqa
    if 'nc' not in _cache:
        _cache['nc']=build()
    nc=_cache['nc']
    fold=_fold(inputs); shards=_shard(inputs)
    in_maps=[]
    for k in range(NCORES):
        m=dict(shards[k]); m.update(fold)
        in_maps.append({kk:np.ascontiguousarray(v,dtype=np.float32) for kk,v in m.items()})
    from concourse.bass_utils import run_bass_kernel_spmd
    res=run_bass_kernel_spmd(nc,in_maps,list(range(NCORES)))
    out=np.empty((N,INTER),np.float32)
    for k in range(NCORES):
        out[k*NB:(k+1)*NB]=res.results[k]['out'].T
    return out


# ---- device program builder (inlined) ----
import numpy as np
import bass_rust
import concourse.bass as bass
import concourse.bacc as bacc
import concourse.mybir as mybir
import concourse.tile as tile

F32=mybir.dt.float32
AF=mybir.ActivationFunctionType
AL=mybir.AluOpType
RG=[list(range(NCORES))]

def mkap(a,offset,dims):
    b=a.copy(); b.offset=offset
    b.ap=bass_rust.VecI64Pair([list(d) for d in dims])
    return b

def build():
    nc=bacc.Bacc("TRN2",target_bir_lowering=False,debug=False,num_devices=NCORES)
    P=lambda n,s: nc.declare_dram_parameter(n,list(s),F32,isOutput=False)
    xT=P('xT',(C,NB)); yT=P('yT',(3,C,NB)); bafT=P('bafT',(C,2*NB))
    gpad=P('gpad',(128,2*WIN)); gpadT=P('gpadT',(QT,128,GC)); osel=P('osel',(NCORES,1))
    xv=P('xvecs',(C,6)); sc=P('sconst',(1,8)); vps=P('vps',(C,3)); vpm=P('vpm',(C,1))
    mgw=P('m_gwT',(C,INTER)); prgw=P('pr_gwT',(3,C,INTER)); bagw=P('ba_gwT',(C,INTER))
    spgw=P('sp_gwT',(9*GC,INTER)); weT=P('w_effT',(2,128,9)); b6=P('bias128',(INTER,6))
    gf=P('gfill',(1,4*INTER)); mgbr=P('mgb_row',(1,INTER))
    out_ext=nc.declare_dram_parameter('out',[INTER,NB],F32,isOutput=True)

    with tile.TileContext(nc) as tc:
      with (tc.tile_pool(name="pp",bufs=1) as pp,
            tc.tile_pool(name="ww",bufs=4) as ww,
            tc.tile_pool(name="dr",bufs=1,space="DRAM") as dr,
            tc.tile_pool(name="ps_or",bufs=1,space="PSUM") as ps_or,
            tc.tile_pool(name="ps_six",bufs=1,space="PSUM") as ps_six,
            tc.tile_pool(name="ps_mid",bufs=2,space="PSUM") as ps_mid,
            tc.tile_pool(name="ps_roll",bufs=2,space="PSUM") as ps_roll,
            tc.tile_pool(name="ps_sm",bufs=1,space="PSUM") as ps_sm):
        dma=nc.sync.dma_start
        ag_in=dr.tile([AGS],F32); ag_out=dr.tile([NCORES*AGS],F32,addr_space='Shared')
        ar_in=dr.tile([ARS],F32); ar_out=dr.tile([ARS],F32,addr_space='Shared')
        p_glob=dr.tile([PGLOB],F32); p_loc=dr.tile([2816],F32)
        def ld(name,shape,src_ap):
            t=pp.tile(shape,F32,tag=name); dma(t[:],src_ap); return t
        xT_s=ld('xT',[128,KT,NB],xT.ap().rearrange("(k p) n -> p k n",p=128))
        yT_s=ld('yT',[128,3,KT,NB],yT.ap().rearrange("j (k p) n -> p j k n",p=128))
        gp_s=pp.tile([128,2,WIN],F32,tag='big',name='gp_s',padded_shape=[128,2,WIN])
        dma(gp_s[:],gpad.ap().rearrange("p (h w) -> p h w",h=2))
        xv_s=ld('xv',[128,KT,6],xv.ap().rearrange("(k p) n -> p k n",p=128))
        vp_s=ld('vp',[128,KT,3],vps.ap().rearrange("(k p) n -> p k n",p=128))
        vpm_s=ld('vpm',[128,KT,1],vpm.ap().rearrange("(k p) n -> p k n",p=128))
        mgw_s=ld('mgw',[128,KT,INTER],mgw.ap().rearrange("(k p) n -> p k n",p=128))
        pr_s=ld('pr',[128,3,KT,INTER],prgw.ap().rearrange("j (k p) n -> p j k n",p=128))
        bag_s=ld('bag',[128,KT,INTER],bagw.ap().rearrange("(k p) n -> p k n",p=128))
        spg_s=ld('spg',[128,18,INTER],spgw.ap().rearrange("(k p) n -> p k n",p=128))
        we_s=ld('we',[128,2,9],weT.ap().rearrange("h p n -> p h n"))
        b6_s=ld('b6',[INTER,6],b6.ap()); gf_s=ld('gf',[1,4*INTER],gf.ap())
        sc_s=ld('sc',[1,8],sc.ap()); mgbr_s=ld('mgbr',[1,INTER],mgbr.ap())
        osel_s=ld('osel',[NCORES,1],osel.ap())
        ones_c=pp.tile([128,1],F32,tag='ones_c'); nc.vector.memset(ones_c[:],1.0)
        zz=pp.tile([128,135],F32,tag='zz'); nc.vector.memset(zz[:],0.0)
        ONESR=gf_s[0:1,3*INTER:4*INTER]
        # conv -> b_s own rows
        outc=pp.tile([9,WIN],F32,tag='outc')
        for ch in range(5):
            pc=ps_mid.tile([128,512],F32,tag='mid')
            for h in range(2):
                nc.tensor.matmul(pc[:9,:468],we_s[:,h,:],gp_s[:,h,ch*468:(ch+1)*468],
                                 start=(h==0),stop=(h==1))
            nc.scalar.activation(outc[:,ch*468:(ch+1)*468],pc[:9,:468],AF.Copy)
        ov=outc[:].rearrange("p (h w) -> p h w",w=PW)
        bsa=pp.tile([HR,128],F32,tag='bsa')
        for m in range(9):
            kh,kw=divmod(m,3)
            bt=ww.tile([HR,128],F32,tag='bt')
            nc.sync.dma_start(bt[:],ov[m:m+1,kh:kh+HR,kw:kw+128])
            if m==0: nc.vector.tensor_copy(bsa[:],bt[:])
            else: nc.vector.tensor_tensor(bsa[:],bsa[:],bt[:],AL.add)
        dma(ag_in[NB*(INTER+1):AGS],bsa[:])
        # psum6
        p6=ps_six.tile([6,512],F32,tag='six')
        for kt in range(KT):
            nc.tensor.matmul(p6[:,:],xv_s[:,kt,:],xT_s[:,kt,:],start=(kt==0),
                             stop=(kt==KT-1))
        p6sb=pp.tile([6,512],F32,tag='p6sb')
        nc.scalar.activation(p6sb[:],p6[:,:],AF.Copy)
        p6r=[]
        for r in range(6):
            t=pp.tile([1,512],F32,tag=f'p6r{r}',name=f'p6r{r}')
            dma(t[:],p6sb[r:r+1,:]); p6r.append(t)
        s_sbs=[]
        for j in range(3):
            s_sbs.append(pp.tile([1,512],F32,tag=f's_sb{j}',name=f's_sb{j}'))
            psv=ps_mid.tile([128,512],F32,tag='mid')
            for kt in range(KT):
                nc.tensor.matmul(psv[:1,:],vp_s[:,kt,j:j+1],yT_s[:,j,kt,:],
                                 start=(kt==0),stop=(kt==KT-1))
            spre=ww.tile([1,512],F32,tag='spre',bufs=1)
            nc.vector.tensor_scalar(spre[:],psv[:1,:],sc_s[0:1,1+j:2+j],None,AL.add)
            t2=ww.tile([1,512],F32,tag='t2',bufs=1)
            nc.vector.tensor_tensor(t2[:],p6r[1+j][:],spre[:],AL.add)
            nc.scalar.activation(s_sbs[j][:],t2[:],AF.Relu)
        b_sb=pp.tile([1,512],F32,tag='b_sb')
        nc.vector.tensor_scalar(b_sb[:],p6r[5][:],sc_s[0:1,5:6],None,AL.add)
        dma(ag_in[NB*INTER:NB*(INTER+1)],b_sb[:])
        a_sb=pp.tile([1,512],F32,tag='a_sb')
        nc.vector.tensor_scalar(a_sb[:],p6r[0][:],sc_s[0:1,0:1],None,AL.add)
        # g_x row-major
        gxo=pp.tile([128,4,INTER],F32,tag='gxo')
        for i4 in range(4):
            pg=ps_mid.tile([128,512],F32,tag='mid')
            for kt in range(KT):
                nc.tensor.matmul(pg[:,:INTER],xT_s[:,kt,i4*128:(i4+1)*128],mgw_s[:,kt,:],
                                 start=(kt==0),stop=False,skip_group_check=True)
            nc.tensor.matmul(pg[:,:INTER],ONESR,mgbr_s[:],start=False,stop=True,
                             skip_group_check=True)
            nc.scalar.activation(gxo[:,i4,:],pg[:,:INTER],AF.Copy)
        dma(mkap(ag_in[:],0,[(128,128),(16384,4),(1,128)]),gxo[:])
        nc.gpsimd.collective_compute("AllGather",AL.bypass,ins=[ag_in[:].opt()],
                                     outs=[ag_out[:].opt()],replica_groups=RG)
        # softmax + p windows
        bs_f=pp.tile([128,128],F32,tag='bs_f')
        for c in range(NCORES):
            dma(bs_f[c*HR:(c+1)*HR,:],ag_out[c*AGS+NB*(INTER+1):c*AGS+AGS])
        e_sb=pp.tile([128,128],F32,tag='e_sb'); zc=pp.tile([128,1],F32,tag='zc')
        nc.scalar.activation(e_sb[:],bs_f[:],AF.Exp,accum_out=zc[:])
        pz=ps_sm.tile([128,512],F32,tag='sm')
        nc.tensor.matmul(pz[:1,:1],zc[:],ones_c[:],start=True,stop=True)
        z_sb=pp.tile([1,1],F32,tag='z_sb'); nc.vector.tensor_copy(z_sb[:],pz[:1,:1])
        zr=pp.tile([1,1],F32,tag='zr'); nc.vector.reciprocal(zr[:],z_sb[:])
        pzb=ps_sm.tile([128,512],F32,tag='sm')
        nc.tensor.matmul(pzb[:,:1],ONESR,zr[:],start=True,stop=True)
        zrb=pp.tile([128,1],F32,tag='zrb'); nc.vector.tensor_copy(zrb[:],pzb[:,:1])
        dma(p_glob[:],zz[:])
        dma(mkap(p_glob[:],262,[(130,128),(1,128)]),e_sb[:])
        p8=pp.tile([NCORES,2048],F32,tag='p8')
        dma(p8[:],mkap(p_glob[:],262,[(HR*PW,NCORES),(PW,HR),(1,128)]))
        ow_sb=pp.tile([1,2048],F32,tag='ow_sb')
        for ch in range(4):
            pwc=ps_sm.tile([128,512],F32,tag='sm')
            nc.tensor.matmul(pwc[:1,:512],osel_s[:],p8[:,ch*512:(ch+1)*512],
                             start=True,stop=True)
            nc.scalar.activation(ow_sb[:,ch*512:(ch+1)*512],pwc[:1,:512],AF.Copy)
        dma(p_loc[:],zz[:,:22])
        dma(mkap(p_loc[:],262,[(PW,HR),(1,128)]),ow_sb[:])
        # q matmuls
        pq=ps_mid.tile([128,512],F32,tag='mid')
        for t in range(QT):
            lq=ww.tile([128,9],F32,tag='lq')
            dma(lq[:],mkap(p_loc[:],128*t,[(1,128),(130,3),(1,3)]))
            gptt=ww.tile([128,GC],F32,tag='gptt',name=f'gptt{t}',bufs=2)
            dma(gptt[:],gpadT.ap()[t])
            nc.tensor.matmul(pq[:9,:GC],lq[:],gptt[:],start=(t==0),stop=(t==QT-1))
        q_sb=pp.tile([9,GC],F32,tag='q_sb')
        nc.scalar.activation(q_sb[:],pq[:9,:GC],AF.Copy)
        dma(ar_in[0:9*GC],q_sb[:])
        # colsums
        cs_sb=pp.tile([128,KT],F32,tag='cs_sb')
        bafv=bafT.ap().rearrange("(k p) n -> p k n",p=128)
        for kt in range(KT):
            bft=ww.tile([128,2*NB],F32,tag='bft',name=f'bft{kt}',bufs=2)
            dma(bft[:],bafv[:,kt,:])
            nc.vector.tensor_reduce(cs_sb[:,kt:kt+1],bft[:],
                                    axis=mybir.AxisListType.X,op=AL.add)
        dma(ar_in[9*GC:ARS],cs_sb[:].rearrange("p k -> k p"))
        nc.gpsimd.collective_compute("AllReduce",AL.add,ins=[ar_in[:].opt()],
                                     outs=[ar_out[:].opt()],replica_groups=RG)
        # post-AR small matvecs
        pgm=ps_sm.tile([128,512],F32,tag='sm2')
        ppm=ps_sm.tile([128,512],F32,tag='sm')
        for kt in range(KT):
            cst=ww.tile([128,1],F32,tag='cst')
            dma(cst[:],ar_out[9*GC+128*kt:9*GC+128*(kt+1)])
            nc.tensor.matmul(pgm[:,:1],bag_s[:,kt,:],cst[:],start=(kt==0),
                             stop=(kt==KT-1),skip_group_check=True)
            nc.tensor.matmul(ppm[:1,:1],vpm_s[:,kt,:],cst[:],start=(kt==0),
                             stop=(kt==KT-1),skip_group_check=True)
        gm_sb=pp.tile([128,1],F32,tag='gm_sb')
        nc.vector.tensor_scalar(gm_sb[:],pgm[:,:1],b6_s[:,4:5],None,AL.add)
        pm_sb=pp.tile([1,1],F32,tag='pm_sb')
        nc.vector.tensor_scalar(pm_sb[:],ppm[:1,:1],sc_s[0:1,4:5],None,AL.add)
        pv=ps_sm.tile([128,512],F32,tag='sm2')
        for t in range(18):
            qrt=ww.tile([128,1],F32,tag='qrt')
            dma(qrt[:],ar_out[128*t:128*(t+1)])
            nc.tensor.matmul(pv[:,:1],spg_s[:,t,:],qrt[:],start=(t==0),stop=(t==17))
        v_sb=pp.tile([128,1],F32,tag='v_sb')
        nc.vector.tensor_scalar(v_sb[:],pv[:,:1],zrb[:],b6_s[:,5:6],AL.mult,AL.add)
        # s_ba + broadcasts + pair terms
        sba=pp.tile([1,512],F32,tag='sba')
        nc.scalar.activation(sba[:],p6r[4][:],AF.Relu,bias=pm_sb[0:1,0:1])
        acc=pp.tile([128,512],F32,tag='acc')
        tmp=pp.tile([128,512],F32,tag='tmp')
        for j in range(3):
            py=ps_roll.tile([128,512],F32,tag='roll')
            for kt in range(KT):
                nc.tensor.matmul(py[:,:],pr_s[:,j,kt,:],yT_s[:,j,kt,:],
                                 start=(kt==0),stop=(kt==KT-1))
            gy=ww.tile([128,512],F32,tag='gy',bufs=1)
            nc.vector.tensor_scalar(gy[:],py[:,:],b6_s[:,1+j:2+j],None,AL.add)
            pb=ps_roll.tile([128,512],F32,tag='roll')
            nc.tensor.matmul(pb[:,:],gf_s[0:1,j*INTER:(j+1)*INTER],s_sbs[j][:],
                             start=True,stop=True)
            if j==0:
                nc.vector.tensor_tensor(acc[:],gy[:],pb[:,:],AL.mult)
            else:
                nc.vector.tensor_tensor(tmp[:],gy[:],pb[:,:],AL.mult)
                nc.vector.tensor_tensor(acc[:],acc[:],tmp[:],AL.add)
        psb=ps_roll.tile([128,512],F32,tag='roll')
        nc.tensor.matmul(psb[:,:],ONESR,sba[:],start=True,stop=True)
        nc.vector.tensor_scalar(tmp[:],psb[:,:],gm_sb[:],None,AL.mult)
        nc.vector.tensor_tensor(acc[:],acc[:],tmp[:],AL.add)
        pab=ps_roll.tile([128,512],F32,tag='roll')
        nc.tensor.matmul(pab[:,:],ONESR,a_sb[:],start=True,stop=True)
        ab_sb=pp.tile([128,512],F32,tag='ab_sb')
        nc.scalar.activation(ab_sb[:],pab[:,:],AF.Copy)
        # gx readback + origin loop
        gx_sb=pp.tile([128,32,128],F32,tag='big',name='gx_sb')
        for c in range(NCORES):
            dma(gx_sb[:,4*c:4*(c+1),:],mkap(ag_out[:],c*AGS,[(128,128),(16384,4),(1,128)]))
        po=ps_or.tile([128,512],F32,tag='orig')
        for jt in range(32):
            cc,lt=divmod(jt,4)
            bc=ww.tile([128,1],F32,tag='bc')
            dma(bc[:],ag_out[cc*AGS+NB*INTER+lt*128:cc*AGS+NB*INTER+(lt+1)*128])
            fT=ww.tile([128,512],F32,tag='fT',bufs=2)
            if jt%8<3:
                nc.scalar.activation(fT[:],ab_sb[:],AF.Relu,bias=bc[:])
            else:
                nc.vector.tensor_scalar(fT[:],ab_sb[:],bc[:],0.0,AL.add,AL.max)
            nc.tensor.matmul(po[:,:],gx_sb[:,jt,:],fT[:],start=(jt==0),stop=(jt==31))
        ot=pp.tile([128,512],F32,tag='ot')
        nc.vector.tensor_scalar(ot[:],po[:,:],1.0/N,v_sb[:],AL.mult,AL.add)
        fin=pp.tile([128,512],F32,tag='fin')
        nc.vector.tensor_tensor(fin[:],acc[:],ot[:],AL.add)
        dma(out_ext.ap(),fin[:])
    nc.compile()
    return nc



# revision 17
# speedup vs baseline: 1.1816x; 1.1816x over previous
import numpy as np

N=4096; C=1024; INTER=128; R=128; RR=R*R; GC=256; NCORES=8; NB=N//NCORES
PW=130; HR=R//NCORES           # 16 output h-rows per core
WINR=HR+2                      # 18 padded rows in window
WIN=WINR*PW                    # 2340
QT=(WIN+127)//128              # 19 k-tiles for q
QPAD=QT*128                    # 2432
PWIN=2694                      # window read span
PGLOB=17280                    # padded p buffer (guard 131 + 16900 + tail)
AGS=NB*(INTER+1)+HR*R          # 512*129+2048 = 68096
ARS=9*GC+C                     # 2304+1024 = 3328
KT=C//128                      # 8

_cache = {}

def _fold(p):
    f32=np.float32
    out={}
    mcw1=p['m_cw'][:INTER]; mcw2=p['m_cw'][INTER:]
    xv=np.zeros((C,6),f32); sc=np.zeros((1,8),f32)
    xv[:,0]=p['m_tw'].T@mcw1; sc[0,0]=p['m_tb']@mcw1            # a
    for j in range(3):
        c1=p['pr_cw'][j,:INTER]; c2=p['pr_cw'][j,INTER:]
        xv[:,1+j]=p['pr_tw'][j].T@c1
        sc[0,1+j]=p['pr_tb'][j]@c1+p['pr_pb'][j]@c2
    xv[:,4]=p['ba_tw'].T@p['ba_cw'][:INTER]
    xv[:,5]=p['m_pw'].T@mcw2; sc[0,5]=p['m_pb']@mcw2            # b
    sc[0,4]=p['ba_tb']@p['ba_cw'][:INTER]+p['ba_pb']@p['ba_cw'][INTER:]
    out['xvecs']=xv; out['sconst']=sc
    vps=np.stack([p['pr_pw'][j].T@p['pr_cw'][j,INTER:] for j in range(3)],1)
    out['vps']=vps.astype(f32)                                   # [C,3]
    out['vpm']=(p['ba_pw'].T@p['ba_cw'][INTER:]/ (2*N)).astype(f32)[:,None]  # [C,1]
    out['m_gwT']=p['m_gw'].T.copy()                              # [C,128]
    out['pr_gwT']=np.stack([p['pr_gw'][j].T for j in range(3)])  # [3,C,128]
    bg=float(p['ba_g'][0])
    out['ba_gwT']=(bg*p['ba_gw'].T/(2*N)).copy()                 # [C,128]
    sg=float(p['sp_g'][0])
    # sp_gwT: [(mh*3+mw)*GC+ic, oc] with (kh,kw)=(2-mh,2-mw), scaled by sp_g
    g=np.transpose(p['sp_gw'],(2,3,1,0))[::-1,::-1]              # [kh',kw',ic,oc] reversed
    out['sp_gwT']=np.ascontiguousarray(sg*g.reshape(9*GC,INTER))
    # w_effT [2,128,9]: w_eff[ic,kh,kw]=sum_c spcw2[c]*sp_pw[c,ic,kh,kw]
    we=np.einsum('c,cikl->ikl',p['sp_cw'][INTER:],p['sp_pw'])    # [GC,3,3]
    out['w_effT']=we.reshape(2,128,9).astype(f32)
    # biases128 [128,6]: m_gb, pr_gb0..2, ba_g*ba_gb(gm bias), sp_g*sp_gb(v bias)
    b6=np.zeros((INTER,6),f32)
    b6[:,0]=p['m_gb']; b6[:,1:4]=p['pr_gb'].T; b6[:,4]=bg*p['ba_gb']; b6[:,5]=sg*p['sp_gb']
    out['bias128']=b6
    gf=np.zeros((1,4*INTER),f32)
    for j in range(3): gf[0,j*INTER:(j+1)*INTER]=p['pr_g'][j]
    gf[0,3*INTER:]=1.0
    out['gfill']=gf
    out['mgb_row']=p['m_gb'][None,:].astype(f32)                 # [1,128] K=1 bias trick
    return out

def _shard(p):
    f32=np.float32
    gpadded=np.pad(p['global_feature'][0],((0,0),(1,1),(1,1)))   # [GC,130,130]
    ins=[]
    for k in range(NCORES):
        d={}
        rs=slice(k*NB,(k+1)*NB)
        d['xT']=np.ascontiguousarray(p['origin_feature'][rs].T)
        yt=np.stack([np.ascontiguousarray(t[rs].T) for t in
                     (p['local_feature'],p['bef_l'],p['aft_l'])])
        d['yT']=yt                                               # [3,C,NB]
        d['bafT']=np.ascontiguousarray(np.concatenate(
            [p['bef'][rs],p['aft'][rs]],0).T)                    # [C,2NB]
        gw=gpadded[:,k*HR:k*HR+WINR,:]                           # [GC,18,130]
        d['gpad']=np.ascontiguousarray(gw.reshape(2,128,WINR*PW)
                    .transpose(1,0,2).reshape(128,2*WINR*PW))
        gt=np.zeros((QPAD,GC),f32)
        gt[:WIN]=gw.reshape(GC,WIN).T
        d['gpadT']=gt.reshape(QT,128,GC)
        osel=np.zeros((NCORES,1),f32); osel[k,0]=1.0
        d['osel']=osel
        ins.append(d)
    return ins

BF16_KEYS={'xT','yT','bafT','gpad','xvecs','vps','m_gwT','pr_gwT',
           'w_effT','gfill','mgb_row'}

def _in_maps(inputs):
    import ml_dtypes
    bf16=ml_dtypes.bfloat16
    fold=_fold(inputs); shards=_shard(inputs)
    in_maps=[]
    for k in range(NCORES):
        m=dict(shards[k]); m.update(fold)
        in_maps.append({kk:np.ascontiguousarray(
            np.asarray(v,dtype=np.float32).astype(bf16) if kk in BF16_KEYS
            else np.asarray(v,dtype=np.float32)) for kk,v in m.items()})
    return in_maps

def kernel(**inputs):
    import ml_dtypes  # noqa
    if 'nc' not in _cache:
        _cache['nc']=build()
    nc=_cache['nc']
    in_maps=_in_maps(inputs)
    from concourse.bass_utils import run_bass_kernel_spmd
    res=run_bass_kernel_spmd(nc,in_maps,list(range(NCORES)))
    out=np.empty((N,INTER),np.float32)
    for k in range(NCORES):
        out[k*NB:(k+1)*NB]=res.results[k]['out'].T
    return out


# ---- device program builder (inlined) ----
import numpy as np
import bass_rust
import concourse.bass as bass
import concourse.bacc as bacc
import concourse.mybir as mybir
import concourse.tile as tile

F32=mybir.dt.float32
BF16=mybir.dt.bfloat16
AF=mybir.ActivationFunctionType
AL=mybir.AluOpType
RG=[list(range(NCORES))]

def mkap(a,offset,dims):
    b=a.copy(); b.offset=offset
    b.ap=bass_rust.VecI64Pair([list(d) for d in dims])
    return b

def build():
    nc=bacc.Bacc("TRN2",target_bir_lowering=False,debug=False,num_devices=NCORES)
    P=lambda n,s,dt=F32: nc.declare_dram_parameter(n,list(s),dt,isOutput=False)
    xT=P('xT',(C,NB),BF16); yT=P('yT',(3,C,NB),BF16); bafT=P('bafT',(C,2*NB),BF16)
    gpad=P('gpad',(128,2*WIN),BF16); gpadT=P('gpadT',(QT,128,GC)); osel=P('osel',(NCORES,1))
    xv=P('xvecs',(C,6),BF16); sc=P('sconst',(1,8)); vps=P('vps',(C,3),BF16); vpm=P('vpm',(C,1))
    mgw=P('m_gwT',(C,INTER),BF16); prgw=P('pr_gwT',(3,C,INTER),BF16); bagw=P('ba_gwT',(C,INTER))
    spgw=P('sp_gwT',(9*GC,INTER)); weT=P('w_effT',(2,128,9),BF16); b6=P('bias128',(INTER,6))
    gf=P('gfill',(1,4*INTER),BF16); mgbr=P('mgb_row',(1,INTER),BF16)
    out_ext=nc.declare_dram_parameter('out',[INTER,NB],F32,isOutput=True)

    with tile.TileContext(nc) as tc:
      with (tc.tile_pool(name="pp",bufs=1) as pp,
            tc.tile_pool(name="ww",bufs=4) as ww,
            tc.tile_pool(name="dr",bufs=1,space="DRAM") as dr,
            tc.tile_pool(name="ps_or",bufs=1,space="PSUM") as ps_or,
            tc.tile_pool(name="ps_six",bufs=1,space="PSUM") as ps_six,
            tc.tile_pool(name="ps_mid",bufs=2,space="PSUM") as ps_mid,
            tc.tile_pool(name="ps_roll",bufs=2,space="PSUM") as ps_roll,
            tc.tile_pool(name="ps_sm",bufs=1,space="PSUM") as ps_sm):
        dma=nc.sync.dma_start
        ag_in=dr.tile([AGS],BF16); ag_out=dr.tile([NCORES*AGS],BF16,addr_space='Shared')
        ar_in=dr.tile([ARS],F32); ar_out=dr.tile([ARS],F32,addr_space='Shared')
        p_glob=dr.tile([PGLOB],F32); p_loc=dr.tile([2816],F32)
        def ld(name,shape,src_ap,dt=F32):
            t=pp.tile(shape,dt,tag=name); dma(t[:],src_ap); return t
        xT_s=ld('xT',[128,KT,NB],xT.ap().rearrange("(k p) n -> p k n",p=128),BF16)
        yT_s=ld('yT',[128,3,KT,NB],yT.ap().rearrange("j (k p) n -> p j k n",p=128),BF16)
        gp_s=pp.tile([128,2,WIN],BF16,tag='big',name='gp_s',padded_shape=[128,2,WIN])
        dma(gp_s[:],gpad.ap().rearrange("p (h w) -> p h w",h=2))
        xv_s=ld('xv',[128,KT,6],xv.ap().rearrange("(k p) n -> p k n",p=128),BF16)
        vp_s=ld('vp',[128,KT,3],vps.ap().rearrange("(k p) n -> p k n",p=128),BF16)
        vpm_s=ld('vpm',[128,KT,1],vpm.ap().rearrange("(k p) n -> p k n",p=128))
        mgw_s=ld('mgw',[128,KT,INTER],mgw.ap().rearrange("(k p) n -> p k n",p=128),BF16)
        pr_s=ld('pr',[128,3,KT,INTER],prgw.ap().rearrange("j (k p) n -> p j k n",p=128),BF16)
        bag_s=ld('bag',[128,KT,INTER],bagw.ap().rearrange("(k p) n -> p k n",p=128))
        spg_s=ld('spg',[128,18,INTER],spgw.ap().rearrange("(k p) n -> p k n",p=128))
        we_s=ld('we',[128,2,9],weT.ap().rearrange("h p n -> p h n"),BF16)
        b6_s=ld('b6',[INTER,6],b6.ap()); gf_s=ld('gf',[1,4*INTER],gf.ap(),BF16)
        sc_s=ld('sc',[1,8],sc.ap()); mgbr_s=ld('mgbr',[1,INTER],mgbr.ap(),BF16)
        osel_s=ld('osel',[NCORES,1],osel.ap())
        ones_c=pp.tile([128,1],F32,tag='ones_c'); nc.vector.memset(ones_c[:],1.0)
        zz=pp.tile([128,135],F32,tag='zz'); nc.vector.memset(zz[:],0.0)
        ONESR=gf_s[0:1,3*INTER:4*INTER]
        # conv -> b_s own rows
        outc=pp.tile([9,WIN],F32,tag='outc')
        for ch in range(5):
            pc=ps_mid.tile([128,512],F32,tag='mid')
            for h in range(2):
                nc.tensor.matmul(pc[:9,:468],we_s[:,h,:],gp_s[:,h,ch*468:(ch+1)*468],
                                 start=(h==0),stop=(h==1))
            nc.scalar.activation(outc[:,ch*468:(ch+1)*468],pc[:9,:468],AF.Copy)
        ov=outc[:].rearrange("p (h w) -> p h w",w=PW)
        bsa=pp.tile([HR,128],F32,tag='bsa')
        for m in range(9):
            kh,kw=divmod(m,3)
            bt=ww.tile([HR,128],F32,tag='bt')
            nc.sync.dma_start(bt[:],ov[m:m+1,kh:kh+HR,kw:kw+128])
            if m==0: nc.vector.tensor_copy(bsa[:],bt[:])
            else: nc.vector.tensor_tensor(bsa[:],bsa[:],bt[:],AL.add)
        bsa16=pp.tile([HR,128],BF16,tag='bsa16')
        nc.vector.tensor_copy(bsa16[:],bsa[:])
        dma(ag_in[NB*(INTER+1):AGS],bsa16[:])
        # psum6
        p6=ps_six.tile([6,512],F32,tag='six')
        for kt in range(KT):
            nc.tensor.matmul(p6[:,:],xv_s[:,kt,:],xT_s[:,kt,:],start=(kt==0),
                             stop=(kt==KT-1))
        p6sb=pp.tile([6,512],F32,tag='p6sb')
        nc.scalar.activation(p6sb[:],p6[:,:],AF.Copy)
        p6r=[]
        for r in range(6):
            t=pp.tile([1,512],F32,tag=f'p6r{r}',name=f'p6r{r}')
            dma(t[:],p6sb[r:r+1,:]); p6r.append(t)
        s_sbs=[]
        for j in range(3):
            s_sbs.append(pp.tile([1,512],BF16,tag=f's_sb{j}',name=f's_sb{j}'))
            psv=ps_mid.tile([128,512],F32,tag='mid')
            for kt in range(KT):
                nc.tensor.matmul(psv[:1,:],vp_s[:,kt,j:j+1],yT_s[:,j,kt,:],
                                 start=(kt==0),stop=(kt==KT-1))
            spre=ww.tile([1,512],F32,tag='spre',bufs=1)
            nc.vector.tensor_scalar(spre[:],psv[:1,:],sc_s[0:1,1+j:2+j],None,AL.add)
            t2=ww.tile([1,512],F32,tag='t2',bufs=1)
            nc.vector.tensor_tensor(t2[:],p6r[1+j][:],spre[:],AL.add)
            nc.scalar.activation(s_sbs[j][:],t2[:],AF.Relu)
        b_sb=pp.tile([1,512],BF16,tag='b_sb')
        nc.vector.tensor_scalar(b_sb[:],p6r[5][:],sc_s[0:1,5:6],None,AL.add)
        dma(ag_in[NB*INTER:NB*(INTER+1)],b_sb[:])
        a_sb=pp.tile([1,512],BF16,tag='a_sb')
        nc.vector.tensor_scalar(a_sb[:],p6r[0][:],sc_s[0:1,0:1],None,AL.add)
        # g_x row-major
        gxo=pp.tile([128,4,INTER],BF16,tag='gxo')
        for i4 in range(4):
            pg=ps_mid.tile([128,512],F32,tag='mid')
            for kt in range(KT):
                nc.tensor.matmul(pg[:,:INTER],xT_s[:,kt,i4*128:(i4+1)*128],mgw_s[:,kt,:],
                                 start=(kt==0),stop=False,skip_group_check=True)
            nc.tensor.matmul(pg[:,:INTER],ONESR,mgbr_s[:],start=False,stop=True,
                             skip_group_check=True)
            nc.scalar.activation(gxo[:,i4,:],pg[:,:INTER],AF.Copy)
        dma(mkap(ag_in[:],0,[(128,128),(16384,4),(1,128)]),gxo[:])
        nc.gpsimd.collective_compute("AllGather",AL.bypass,ins=[ag_in[:].opt()],
                                     outs=[ag_out[:].opt()],replica_groups=RG)
        # softmax + p windows
        bs_f=pp.tile([128,128],BF16,tag='bs_f')
        for c in range(NCORES):
            dma(bs_f[c*HR:(c+1)*HR,:],ag_out[c*AGS+NB*(INTER+1):c*AGS+AGS])
        e_sb=pp.tile([128,128],F32,tag='e_sb'); zc=pp.tile([128,1],F32,tag='zc')
        nc.scalar.activation(e_sb[:],bs_f[:],AF.Exp,accum_out=zc[:])
        pz=ps_sm.tile([128,512],F32,tag='sm')
        nc.tensor.matmul(pz[:1,:1],zc[:],ones_c[:],start=True,stop=True)
        z_sb=pp.tile([1,1],F32,tag='z_sb'); nc.vector.tensor_copy(z_sb[:],pz[:1,:1])
        zr=pp.tile([1,1],F32,tag='zr'); nc.vector.reciprocal(zr[:],z_sb[:])
        zr16=pp.tile([1,1],BF16,tag='zr16'); nc.vector.tensor_copy(zr16[:],zr[:])
        pzb=ps_sm.tile([128,512],F32,tag='sm')
        nc.tensor.matmul(pzb[:,:1],ONESR,zr16[:],start=True,stop=True)
        zrb=pp.tile([128,1],F32,tag='zrb'); nc.vector.tensor_copy(zrb[:],pzb[:,:1])
        dma(p_glob[:],zz[:])
        dma(mkap(p_glob[:],262,[(130,128),(1,128)]),e_sb[:])
        p8=pp.tile([NCORES,2048],F32,tag='p8')
        dma(p8[:],mkap(p_glob[:],262,[(HR*PW,NCORES),(PW,HR),(1,128)]))
        ow_sb=pp.tile([1,2048],F32,tag='ow_sb')
        for ch in range(4):
            pwc=ps_sm.tile([128,512],F32,tag='sm')
            nc.tensor.matmul(pwc[:1,:512],osel_s[:],p8[:,ch*512:(ch+1)*512],
                             start=True,stop=True)
            nc.scalar.activation(ow_sb[:,ch*512:(ch+1)*512],pwc[:1,:512],AF.Copy)
        dma(p_loc[:],zz[:,:22])
        dma(mkap(p_loc[:],262,[(PW,HR),(1,128)]),ow_sb[:])
        # q matmuls
        pq=ps_mid.tile([128,512],F32,tag='mid')
        for t in range(QT):
            lq=ww.tile([128,9],F32,tag='lq')
            dma(lq[:],mkap(p_loc[:],128*t,[(1,128),(130,3),(1,3)]))
            gptt=ww.tile([128,GC],F32,tag='gptt',name=f'gptt{t}',bufs=2)
            dma(gptt[:],gpadT.ap()[t])
            nc.tensor.matmul(pq[:9,:GC],lq[:],gptt[:],start=(t==0),stop=(t==QT-1))
        q_sb=pp.tile([9,GC],F32,tag='q_sb')
        nc.scalar.activation(q_sb[:],pq[:9,:GC],AF.Copy)
        dma(ar_in[0:9*GC],q_sb[:])
        # colsums
        cs_sb=pp.tile([128,KT],F32,tag='cs_sb')
        bafv=bafT.ap().rearrange("(k p) n -> p k n",p=128)
        for kt in range(KT):
            bft=ww.tile([128,2*NB],BF16,tag='bft',name=f'bft{kt}',bufs=2)
            dma(bft[:],bafv[:,kt,:])
            nc.vector.tensor_reduce(cs_sb[:,kt:kt+1],bft[:],
                                    axis=mybir.AxisListType.X,op=AL.add)
        dma(ar_in[9*GC:ARS],cs_sb[:].rearrange("p k -> k p"))
        nc.gpsimd.collective_compute("AllReduce",AL.add,ins=[ar_in[:].opt()],
                                     outs=[ar_out[:].opt()],replica_groups=RG)
        # post-AR small matvecs
        pgm=ps_sm.tile([128,512],F32,tag='sm2')
        ppm=ps_sm.tile([128,512],F32,tag='sm')
        for kt in range(KT):
            cst=ww.tile([128,1],F32,tag='cst')
            dma(cst[:],ar_out[9*GC+128*kt:9*GC+128*(kt+1)])
            nc.tensor.matmul(pgm[:,:1],bag_s[:,kt,:],cst[:],start=(kt==0),
                             stop=(kt==KT-1),skip_group_check=True)
            nc.tensor.matmul(ppm[:1,:1],vpm_s[:,kt,:],cst[:],start=(kt==0),
                             stop=(kt==KT-1),skip_group_check=True)
        gm_sb=pp.tile([128,1],F32,tag='gm_sb')
        nc.vector.tensor_scalar(gm_sb[:],pgm[:,:1],b6_s[:,4:5],None,AL.add)
        pm_sb=pp.tile([1,1],F32,tag='pm_sb')
        nc.vector.tensor_scalar(pm_sb[:],ppm[:1,:1],sc_s[0:1,4:5],None,AL.add)
        pv=ps_sm.tile([128,512],F32,tag='sm2')
        for t in range(18):
            qrt=ww.tile([128,1],F32,tag='qrt')
            dma(qrt[:],ar_out[128*t:128*(t+1)])
            nc.tensor.matmul(pv[:,:1],spg_s[:,t,:],qrt[:],start=(t==0),stop=(t==17))
        v_sb=pp.tile([128,1],F32,tag='v_sb')
        nc.vector.tensor_scalar(v_sb[:],pv[:,:1],zrb[:],b6_s[:,5:6],AL.mult,AL.add)
        # s_ba + broadcasts + pair terms
        sba=pp.tile([1,512],BF16,tag='sba')
        nc.scalar.activation(sba[:],p6r[4][:],AF.Relu,bias=pm_sb[0:1,0:1])
        acc=pp.tile([128,512],F32,tag='acc')
        tmp=pp.tile([128,512],F32,tag='tmp')
        for j in range(3):
            py=ps_roll.tile([128,512],F32,tag='roll')
            for kt in range(KT):
                nc.tensor.matmul(py[:,:],pr_s[:,j,kt,:],yT_s[:,j,kt,:],
                                 start=(kt==0),stop=(kt==KT-1))
            gy=ww.tile([128,512],F32,tag='gy',bufs=1)
            nc.vector.tensor_scalar(gy[:],py[:,:],b6_s[:,1+j:2+j],None,AL.add)
            pb=ps_roll.tile([128,512],F32,tag='roll')
            nc.tensor.matmul(pb[:,:],gf_s[0:1,j*INTER:(j+1)*INTER],s_sbs[j][:],
                             start=True,stop=True)
            if j==0:
                nc.vector.tensor_tensor(acc[:],gy[:],pb[:,:],AL.mult)
            else:
                nc.vector.tensor_tensor(tmp[:],gy[:],pb[:,:],AL.mult)
                nc.vector.tensor_tensor(acc[:],acc[:],tmp[:],AL.add)
        psb=ps_roll.tile([128,512],F32,tag='roll')
        nc.tensor.matmul(psb[:,:],ONESR,sba[:],start=True,stop=True)
        nc.vector.tensor_scalar(tmp[:],psb[:,:],gm_sb[:],None,AL.mult)
        nc.vector.tensor_tensor(acc[:],acc[:],tmp[:],AL.add)
        pab=ps_roll.tile([128,512],F32,tag='roll')
        nc.tensor.matmul(pab[:,:],ONESR,a_sb[:],start=True,stop=True)
        ab_sb=pp.tile([128,512],BF16,tag='ab_sb')
        nc.scalar.activation(ab_sb[:],pab[:,:],AF.Copy)
        # gx readback + origin loop
        gx_sb=pp.tile([128,32,128],BF16,tag='big2',name='gx_sb')
        for c in range(NCORES):
            dma(gx_sb[:,4*c:4*(c+1),:],mkap(ag_out[:],c*AGS,[(128,128),(16384,4),(1,128)]))
        po=ps_or.tile([128,512],F32,tag='orig')
        for jt in range(32):
            cc,lt=divmod(jt,4)
            bc=ww.tile([128,1],BF16,tag='bc')
            dma(bc[:],ag_out[cc*AGS+NB*INTER+lt*128:cc*AGS+NB*INTER+(lt+1)*128])
            bc32=ww.tile([128,1],F32,tag='bc32')
            nc.gpsimd.tensor_copy(bc32[:],bc[:])
            fT=ww.tile([128,512],BF16,tag='fT',bufs=2)
            if jt%8<3:
                nc.scalar.activation(fT[:],ab_sb[:],AF.Relu,bias=bc32[:])
            else:
                nc.vector.tensor_scalar(fT[:],ab_sb[:],bc32[:],0.0,AL.add,AL.max)
            nc.tensor.matmul(po[:,:],gx_sb[:,jt,:],fT[:],start=(jt==0),stop=(jt==31))
        ot=pp.tile([128,512],F32,tag='ot')
        nc.vector.tensor_scalar(ot[:],po[:,:],1.0/N,v_sb[:],AL.mult,AL.add)
        fin=pp.tile([128,512],F32,tag='fin')
        nc.vector.tensor_tensor(fin[:],acc[:],ot[:],AL.add)
        dma(out_ext.ap(),fin[:])
    nc.compile()
    return nc



# revision 23
# speedup vs baseline: 2.6251x; 2.2217x over previous
import numpy as np

N=4096; C=1024; INTER=128; R=128; RR=R*R; GC=256; NCORES=8; NB=N//NCORES
PW=130; HR=R//NCORES           # 16 output h-rows per core
WINR=HR+2                      # 18 padded rows in window
WIN=WINR*PW                    # 2340
PADW=2368                      # zero-padded free span per half (max read 2342)
CH=416; NCH=5                  # r-conv chunking: 5*416 = 2080 = HR*130
AGS=NB*(INTER+1)               # 512*129 = 66048 (g_x contiguous-tiled + b row)
ARS=1280                       # 10 blocks of 128: [u, Z-block, colsums k=0..7]
KT=C//128                      # 8

_cache = {}

def _perm(a):
    # [C, m] -> [128, KT*m] so each SBUF partition line is DRAM-contiguous
    m=a.shape[1]
    return np.ascontiguousarray(a.reshape(KT,128,m).transpose(1,0,2).reshape(128,KT*m))

def _fold(p):
    f32=np.float32
    out={}
    mcw1=p['m_cw'][:INTER]; mcw2=p['m_cw'][INTER:]
    xv=np.zeros((C,6),f32); sc=np.zeros((1,8),f32)
    xv[:,0]=p['m_tw'].T@mcw1; sc[0,0]=p['m_tb']@mcw1            # a
    for j in range(3):
        c1=p['pr_cw'][j,:INTER]; c2=p['pr_cw'][j,INTER:]
        xv[:,1+j]=p['pr_tw'][j].T@c1
        sc[0,1+j]=p['pr_tb'][j]@c1+p['pr_pb'][j]@c2
    xv[:,4]=p['ba_tw'].T@p['ba_cw'][:INTER]
    xv[:,5]=p['m_pw'].T@mcw2; sc[0,5]=p['m_pb']@mcw2            # b
    sc[0,4]=p['ba_tb']@p['ba_cw'][:INTER]+p['ba_pb']@p['ba_cw'][INTER:]
    out['xvecs']=_perm(xv); out['sconst']=sc
    vps=np.stack([p['pr_pw'][j].T@p['pr_cw'][j,INTER:] for j in range(3)],1)
    out['vps']=_perm(vps.astype(f32))                            # [128,KT*3]
    out['vpm']=_perm((p['ba_pw'].T@p['ba_cw'][INTER:]/(2*N)).astype(f32)[:,None])
    out['m_gwT']=_perm(p['m_gw'].T.copy())                       # [128,KT*128]
    out['pr_gwT']=np.ascontiguousarray(
        np.stack([p['pr_gw'][j].T for j in range(3)])            # [3,C,128]
        .reshape(3,KT,128,INTER).transpose(2,0,1,3).reshape(128,3*KT*INTER))
    bg=float(p['ba_g'][0])
    out['ba_gwT']=_perm((bg*p['ba_gw'].T/(2*N)).copy())
    sg=float(p['sp_g'][0])
    # r-conv weights: spgr[p, t, oc] with t=(kh*3+kw)*2+h2
    arr=np.zeros((18,128,INTER),f32)
    for kh in range(3):
        for kw in range(3):
            for h2 in range(2):
                t=(kh*3+kw)*2+h2
                arr[t]=sg*p['sp_gw'][:,h2*128:(h2+1)*128,kh,kw].T
    out['spgr']=np.ascontiguousarray(arr.transpose(1,0,2).reshape(128,18*INTER))
    # w_effT [128, 2, 9]: w_eff[ic,kh,kw]=sum_c spcw2[c]*sp_pw[c,ic,kl]
    we=np.einsum('c,cikl->ikl',p['sp_cw'][INTER:],p['sp_pw'])    # [GC,3,3]
    out['w_effT']=np.ascontiguousarray(
        we.reshape(2,128,9).transpose(1,0,2).reshape(128,18)).astype(f32)
    # biases128 [128,6]: m_gb, pr_gb0..2, ba_g*ba_gb(gm bias), sp_g*sp_gb(v bias)
    b6=np.zeros((INTER,6),f32)
    b6[:,0]=p['m_gb']; b6[:,1:4]=p['pr_gb'].T; b6[:,4]=bg*p['ba_gb']; b6[:,5]=sg*p['sp_gb']
    out['bias128']=b6
    gf=np.zeros((1,4*INTER),f32)
    for j in range(3): gf[0,j*INTER:(j+1)*INTER]=p['pr_g'][j]
    gf[0,3*INTER:]=1.0
    out['gfill']=gf
    out['mgb_row']=p['m_gb'][None,:].astype(f32)                 # [1,128] K=1 bias trick
    out['idn']=np.eye(128,dtype=f32)
    return out

def _shard(p):
    f32=np.float32
    gpadded=np.pad(p['global_feature'][0],((0,0),(1,1),(1,1)))   # [GC,130,130]
    ins=[]
    for k in range(NCORES):
        d={}
        rs=slice(k*NB,(k+1)*NB)
        d['xT']=_perm(np.ascontiguousarray(p['origin_feature'][rs].T))
        d['yT']=np.ascontiguousarray(np.stack(
            [np.ascontiguousarray(t[rs].T) for t in
             (p['local_feature'],p['bef_l'],p['aft_l'])])        # [3,C,NB]
            .reshape(3,KT,128,NB).transpose(2,0,1,3).reshape(128,3*KT*NB))
        d['bafT']=_perm(np.ascontiguousarray(np.concatenate(
            [p['bef'][rs],p['aft'][rs]],0).T))                   # [128,KT*2NB]
        gw=gpadded[:,k*HR:k*HR+WINR,:]                           # [GC,18,130]
        gp=np.zeros((128,2,PADW),f32)
        gp[:,:,:WIN]=gw.reshape(2,128,WIN).transpose(1,0,2)
        d['gpad']=gp
        ins.append(d)
    return ins

BF16_KEYS={'xT','yT','bafT','gpad','spgr','xvecs','vps','vpm','m_gwT','pr_gwT',
           'ba_gwT','w_effT','gfill','mgb_row','idn'}

def _in_maps(inputs):
    import ml_dtypes
    bf16=ml_dtypes.bfloat16
    fold=_fold(inputs); shards=_shard(inputs)
    in_maps=[]
    for k in range(NCORES):
        m=dict(shards[k]); m.update(fold)
        mm={}
        for kk,v in m.items():
            v=np.asarray(v,dtype=np.float32)
            if kk in BF16_KEYS: v=v.astype(bf16)
            if kk=='idn':
                mm['idn16']=np.ascontiguousarray(v)
                mm['idn32']=np.ascontiguousarray(np.asarray(m['idn'],dtype=np.float32))
            else:
                mm[kk]=np.ascontiguousarray(v)
        in_maps.append(mm)
    return in_maps

def kernel(**inputs):
    import ml_dtypes  # noqa
    if 'nc' not in _cache:
        _cache['nc']=build()
    nc=_cache['nc']
    in_maps=_in_maps(inputs)
    from concourse.bass_utils import run_bass_kernel_spmd
    res=run_bass_kernel_spmd(nc,in_maps,list(range(NCORES)))
    out=np.empty((N,INTER),np.float32)
    for k in range(NCORES):
        out[k*NB:(k+1)*NB]=res.results[k]['out'].T
    return out


# ---- device program builder (inlined) ----
import numpy as np
import bass_rust
import concourse.bass as bass
import concourse.bacc as bacc
import concourse.mybir as mybir
import concourse.tile as tile

F32=mybir.dt.float32
BF16=mybir.dt.bfloat16
AF=mybir.ActivationFunctionType
AL=mybir.AluOpType
AX=mybir.AxisListType
RG=[list(range(NCORES))]

def mkap(a,offset,dims):
    b=a.copy(); b.offset=offset
    b.ap=bass_rust.VecI64Pair([list(d) for d in dims])
    return b

def build():
    nc=bacc.Bacc("TRN2",target_bir_lowering=False,debug=False,num_devices=NCORES)
    P=lambda n,s,dt=F32: nc.declare_dram_parameter(n,list(s),dt,isOutput=False)
    xv=P('xvecs',(128,KT,6),BF16); xT=P('xT',(128,KT,NB),BF16)
    mgw=P('m_gwT',(128,KT,INTER),BF16); mgbr=P('mgb_row',(1,INTER),BF16)
    gf=P('gfill',(1,4*INTER),BF16); sc=P('sconst',(1,8)); b6=P('bias128',(INTER,6))
    gpad=P('gpad',(128,2,PADW),BF16); weT=P('w_effT',(128,2,9),BF16)
    spgr=P('spgr',(128,18,INTER),BF16)
    yT=P('yT',(128,3,KT,NB),BF16); vps=P('vps',(128,KT,3),BF16)
    prgw=P('pr_gwT',(128,3,KT,INTER),BF16)
    bafT=P('bafT',(128,KT,2*NB),BF16)
    bagw=P('ba_gwT',(128,KT,INTER),BF16); vpm=P('vpm',(128,KT,1),BF16)
    idn16=P('idn16',(128,128),BF16); idn32=P('idn32',(128,128))
    out_ext=nc.declare_dram_parameter('out',[INTER,NB],F32,isOutput=True)

    with tile.TileContext(nc) as tc:
      with (tc.tile_pool(name="pp",bufs=1) as pp,
            tc.tile_pool(name="ww",bufs=4) as ww,
            tc.tile_pool(name="dr",bufs=1,space="DRAM") as dr,
            tc.tile_pool(name="ps_or",bufs=1,space="PSUM") as ps_or,
            tc.tile_pool(name="ps_mid",bufs=2,space="PSUM") as ps_mid,
            tc.tile_pool(name="ps_rc",bufs=2,space="PSUM") as ps_rc,
            tc.tile_pool(name="ps_bt",bufs=1,space="PSUM") as ps_bt,
            tc.tile_pool(name="ps_roll",bufs=2,space="PSUM") as ps_roll):
        dma=nc.sync.dma_start
        ag_in=dr.tile([AGS],BF16); ag_out=dr.tile([NCORES*AGS],BF16,addr_space='Shared')
        ar_in=dr.tile([ARS],F32); ar_out=dr.tile([ARS],F32,addr_space='Shared')
        def ld(name,shape,src_ap,dt=F32):
            t=pp.tile(shape,dt,tag=name); dma(t[:],src_ap); return t
        # priority loads for the early g_x/psum6 phase
        xv_s=ld('xv',[128,KT,6],xv.ap(),BF16)
        xT_s=ld('xT',[128,KT,NB],xT.ap(),BF16)
        mgw_s=ld('mgw',[128,KT,INTER],mgw.ap(),BF16)
        mgbr_s=ld('mgbr',[1,INTER],mgbr.ap(),BF16)
        gf_s=ld('gf',[1,4*INTER],gf.ap(),BF16)
        sc_s=ld('sc',[1,8],sc.ap()); b6_s=ld('b6',[INTER,6],b6.ap())
        # spatial phase loads
        gp_s=ld('gp',[128,2,PADW],gpad.ap(),BF16)
        we_s=ld('we',[128,2,9],weT.ap(),BF16)
        spgr_s=ld('spgr',[128,18,INTER],spgr.ap(),BF16)
        # pair phase loads
        yT_s=ld('yT',[128,3,KT,NB],yT.ap(),BF16)
        vp_s=ld('vp',[128,KT,3],vps.ap(),BF16)
        pr_s=ld('pr',[128,3,KT,INTER],prgw.ap(),BF16)
        bag_s=ld('bag',[128,KT,INTER],bagw.ap(),BF16)
        vpm_s=ld('vpm',[128,KT,1],vpm.ap(),BF16)
        id16_s=ld('id16',[128,128],idn16.ap(),BF16)
        id32_s=ld('id32',[128,128],idn32.ap())
        ones_c=pp.tile([128,1],F32,tag='ones_c'); nc.vector.memset(ones_c[:],1.0)
        ONESR=gf_s[0:1,3*INTER:4*INTER]
        # ---- psum6: 6 score rows from x ----
        p6=ps_mid.tile([6,512],F32,tag='mid')
        for kt in range(KT):
            nc.tensor.matmul(p6[:,:],xv_s[:,kt,:],xT_s[:,kt,:],start=(kt==0),
                             stop=(kt==KT-1))
        p6sb=pp.tile([6,512],F32,tag='p6sb')
        nc.scalar.activation(p6sb[:],p6[:,:],AF.Copy)
        p6r=[]
        for r in range(6):
            t=pp.tile([1,512],F32,tag=f'p6r{r}',name=f'p6r{r}')
            dma(t[:],p6sb[r:r+1,:]); p6r.append(t)
        b_sb=pp.tile([1,512],BF16,tag='b_sb')
        nc.vector.tensor_scalar(b_sb[:],p6r[5][:],sc_s[0:1,5:6],None,AL.add)
        dma(ag_in[NB*INTER:AGS],b_sb[:])
        a_sb=pp.tile([1,512],BF16,tag='a_sb')
        nc.vector.tensor_scalar(a_sb[:],p6r[0][:],sc_s[0:1,0:1],None,AL.add)
        # ---- g_x row blocks [i,e]; ag layout j*512+lt*128+e ----
        gxo=pp.tile([128,4,INTER],BF16,tag='gxo')
        for i4 in range(4):
            pg=ps_mid.tile([128,512],F32,tag='mid')
            for kt in range(KT):
                nc.tensor.matmul(pg[:,:INTER],xT_s[:,kt,i4*128:(i4+1)*128],mgw_s[:,kt,:],
                                 start=(kt==0),stop=False,skip_group_check=True)
            nc.tensor.matmul(pg[:,:INTER],ONESR,mgbr_s[:],start=False,stop=True,
                             skip_group_check=True)
            nc.scalar.activation(gxo[:,i4,:],pg[:,:INTER],AF.Copy)
        dma(mkap(ag_in[:],0,[(512,128),(128,4),(1,128)]),gxo[:])
        nc.gpsimd.collective_compute("AllGather",AL.bypass,ins=[ag_in[:].opt()],
                                     outs=[ag_out[:].opt()],replica_groups=RG)
        # ---- b_s conv (1 channel) -> bsa [HR,128] -> e_row ----
        outc=pp.tile([9,WIN],F32,tag='outc')
        for ch in range(5):
            pc=ps_mid.tile([128,512],F32,tag='mid')
            for h in range(2):
                nc.tensor.matmul(pc[:9,:468],we_s[:,h,:],gp_s[:,h,ch*468:(ch+1)*468],
                                 start=(h==0),stop=(h==1))
            nc.scalar.activation(outc[:,ch*468:(ch+1)*468],pc[:9,:468],AF.Copy)
        ov=outc[:].rearrange("p (h w) -> p h w",w=PW)
        bsa=pp.tile([HR,128],F32,tag='bsa')
        for m in range(9):
            kh,kw=divmod(m,3)
            bt=ww.tile([HR,128],F32,tag='bt')
            nc.sync.dma_start(bt[:],ov[m:m+1,kh:kh+HR,kw:kw+128])
            if m==0: nc.vector.tensor_copy(bsa[:],bt[:])
            else: nc.vector.tensor_tensor(bsa[:],bsa[:],bt[:],AL.add)
        e16=pp.tile([HR,128],BF16,tag='e16')
        zc16=pp.tile([HR,1],F32,tag='zc16')
        nc.scalar.activation(e16[:],bsa[:],AF.Exp,accum_out=zc16[:])
        e_row=pp.tile([1,NCH*CH],BF16,tag='e_row')
        nc.vector.memset(e_row[:],0.0)
        dma(mkap(e_row[:],0,[(1,1),(PW,HR),(1,128)]),e16[:])
        # csu accumulator [128,10]: col0=u, col1=Z(p0), col2+kt=colsums
        csu=pp.tile([128,10],F32,tag='csu')
        nc.vector.memset(csu[:],0.0)
        pz=ps_mid.tile([128,512],F32,tag='mid')
        nc.tensor.matmul(pz[:1,:1],zc16[:],ones_c[:HR,:],start=True,stop=True)
        nc.vector.tensor_copy(csu[0:1,1:2],pz[:1,:1])
        # ---- r-conv chunks + weighted reduce into u ----
        ured=pp.tile([128,NCH],F32,tag='ured')
        for c in range(NCH):
            prp=ps_rc.tile([128,512],F32,tag='rc')
            t=0
            for kh in range(3):
                for kw in range(3):
                    off=kh*PW+kw
                    for h2 in range(2):
                        nc.tensor.matmul(prp[:,:CH],spgr_s[:,t,:],
                                         gp_s[:,h2,c*CH+off:c*CH+off+CH],
                                         start=(t==0),stop=(t==17))
                        t+=1
            ebp=ps_mid.tile([128,512],F32,tag='mid')
            nc.tensor.matmul(ebp[:,:CH],ONESR,e_row[0:1,c*CH:(c+1)*CH],
                             start=True,stop=True)
            eb_sb=ww.tile([128,CH],F32,tag='eb_sb',bufs=2)
            nc.scalar.activation(eb_sb[:],ebp[:,:CH],AF.Copy)
            tmp_rc=ww.tile([128,CH],F32,tag='tmp_rc',bufs=2)
            nc.vector.tensor_tensor(tmp_rc[:],prp[:,:CH],eb_sb[:],AL.mult)
            nc.vector.tensor_reduce(ured[:,c:c+1],tmp_rc[:],axis=AX.X,op=AL.add)
        nc.vector.tensor_reduce(csu[:,0:1],ured[:],axis=AX.X,op=AL.add)
        # ---- pair units ----
        s_sbs=[]
        for j in range(3):
            s_sbs.append(pp.tile([1,512],BF16,tag=f's_sb{j}',name=f's_sb{j}'))
            psv=ps_mid.tile([128,512],F32,tag='mid')
            for kt in range(KT):
                nc.tensor.matmul(psv[:1,:],vp_s[:,kt,j:j+1],yT_s[:,j,kt,:],
                                 start=(kt==0),stop=(kt==KT-1))
            spre=ww.tile([1,512],F32,tag='spre',bufs=1)
            nc.vector.tensor_scalar(spre[:],psv[:1,:],sc_s[0:1,1+j:2+j],None,AL.add)
            t2=ww.tile([1,512],F32,tag='t2',bufs=1)
            nc.vector.tensor_tensor(t2[:],p6r[1+j][:],spre[:],AL.add)
            nc.scalar.activation(s_sbs[j][:],t2[:],AF.Relu)
        acc=pp.tile([128,512],F32,tag='acc')
        tmp=pp.tile([128,512],F32,tag='tmp')
        for j in range(3):
            py=ps_roll.tile([128,512],F32,tag='roll')
            for kt in range(KT):
                nc.tensor.matmul(py[:,:],pr_s[:,j,kt,:],yT_s[:,j,kt,:],
                                 start=(kt==0),stop=(kt==KT-1))
            gy=ww.tile([128,512],F32,tag='gy',bufs=1)
            nc.vector.tensor_scalar(gy[:],py[:,:],b6_s[:,1+j:2+j],None,AL.add)
            pb=ps_roll.tile([128,512],F32,tag='roll')
            nc.tensor.matmul(pb[:,:],gf_s[0:1,j*INTER:(j+1)*INTER],s_sbs[j][:],
                             start=True,stop=True)
            if j==0:
                nc.vector.tensor_tensor(acc[:],gy[:],pb[:,:],AL.mult)
            else:
                nc.vector.tensor_tensor(tmp[:],gy[:],pb[:,:],AL.mult)
                nc.vector.tensor_tensor(acc[:],acc[:],tmp[:],AL.add)
        # ---- colsums of bef/aft ----
        for kt in range(KT):
            bft=ww.tile([128,2*NB],BF16,tag='bft',name=f'bft{kt}',bufs=2)
            dma(bft[:],bafT.ap()[:,kt,:])
            nc.vector.tensor_reduce(csu[:,2+kt:3+kt],bft[:],axis=AX.X,op=AL.add)
        # ---- transpose csu -> [10,128] -> AllReduce ----
        pT=ps_mid.tile([128,512],F32,tag='mid')
        nc.tensor.transpose(pT[:10,:128],csu[:],id32_s[:])
        cs10=pp.tile([10,128],F32,tag='cs10')
        nc.scalar.activation(cs10[:],pT[:10,:128],AF.Copy)
        dma(ar_in[:],cs10[:])
        nc.gpsimd.collective_compute("AllReduce",AL.add,ins=[ar_in[:].opt()],
                                     outs=[ar_out[:].opt()],replica_groups=RG)
        # ---- a broadcast + b transpose + gx readback ----
        pab=ps_roll.tile([128,512],F32,tag='roll')
        nc.tensor.matmul(pab[:,:],ONESR,a_sb[:],start=True,stop=True)
        ab_sb=pp.tile([128,512],BF16,tag='ab_sb')
        nc.scalar.activation(ab_sb[:],pab[:,:],AF.Copy)
        B2=pp.tile([32,128],BF16,tag='B2')
        for cc in range(NCORES):
            dma(B2[4*cc:4*(cc+1),:],ag_out[cc*AGS+NB*INTER:(cc+1)*AGS])
        pBT=ps_bt.tile([128,512],BF16,tag='bt')
        nc.tensor.transpose(pBT[:128,:32],B2[:],id16_s[:32,:32])
        BTf=pp.tile([128,32],F32,tag='BTf')
        nc.scalar.activation(BTf[:],pBT[:128,:32],AF.Copy)
        gx_sb=pp.tile([128,32,128],BF16,tag='gx_sb')
        for cc in range(NCORES):
            dma(gx_sb[:,4*cc:4*(cc+1),:],
                mkap(ag_out[:],cc*AGS,[(512,128),(128,4),(1,128)]))
        # ---- origin loop ----
        po=ps_or.tile([128,512],F32,tag='orig')
        for jt in range(32):
            fT=ww.tile([128,512],BF16,tag='fT',bufs=3)
            if jt%8<3:
                nc.scalar.activation(fT[:],ab_sb[:],AF.Relu,bias=BTf[:,jt:jt+1])
            else:
                nc.vector.tensor_scalar(fT[:],ab_sb[:],BTf[:,jt:jt+1],0.0,AL.add,AL.max)
            nc.tensor.matmul(po[:,:],gx_sb[:,jt,:],fT[:],start=(jt==0),stop=(jt==31))
        # ---- post-AllReduce readback + small matvecs ----
        R10=pp.tile([10,128],F32,tag='R10')
        dma(R10[:],ar_out[:])
        pRT=ps_mid.tile([128,512],F32,tag='mid')
        nc.tensor.transpose(pRT[:128,:10],R10[:],id32_s[:10,:10])
        RT=pp.tile([128,10],F32,tag='RT')
        nc.scalar.activation(RT[:],pRT[:128,:10],AF.Copy)
        RT16=pp.tile([128,10],BF16,tag='RT16')
        nc.vector.tensor_copy(RT16[:],RT[:])
        pgm=ps_mid.tile([128,512],F32,tag='mid')
        ppm=ps_mid.tile([128,512],F32,tag='mid')
        for kt in range(KT):
            nc.tensor.matmul(pgm[:,:1],bag_s[:,kt,:],RT16[:,2+kt:3+kt],start=(kt==0),
                             stop=(kt==KT-1),skip_group_check=True)
            nc.tensor.matmul(ppm[:1,:1],vpm_s[:,kt,:],RT16[:,2+kt:3+kt],start=(kt==0),
                             stop=(kt==KT-1),skip_group_check=True)
        gm_sb=pp.tile([128,1],F32,tag='gm_sb')
        nc.vector.tensor_scalar(gm_sb[:],pgm[:,:1],b6_s[:,4:5],None,AL.add)
        pm_sb=pp.tile([1,1],F32,tag='pm_sb')
        nc.vector.tensor_scalar(pm_sb[:],ppm[:1,:1],sc_s[0:1,4:5],None,AL.add)
        # v = u/Z + bias
        zr=pp.tile([1,1],F32,tag='zr'); nc.vector.reciprocal(zr[:],RT[0:1,1:2])
        zr16=pp.tile([1,1],BF16,tag='zr16'); nc.vector.tensor_copy(zr16[:],zr[:])
        pzb=ps_roll.tile([128,512],F32,tag='roll')
        nc.tensor.matmul(pzb[:,:1],ONESR,zr16[:],start=True,stop=True)
        zrb=pp.tile([128,1],F32,tag='zrb'); nc.vector.tensor_copy(zrb[:],pzb[:,:1])
        v_sb=pp.tile([128,1],F32,tag='v_sb')
        nc.vector.tensor_scalar(v_sb[:],RT[:,0:1],zrb[:],b6_s[:,5:6],AL.mult,AL.add)
        # s_ba row + final combine
        sba=pp.tile([1,512],BF16,tag='sba')
        nc.scalar.activation(sba[:],p6r[4][:],AF.Relu,bias=pm_sb[0:1,0:1])
        psb=ps_roll.tile([128,512],F32,tag='roll')
        nc.tensor.matmul(psb[:,:],ONESR,sba[:],start=True,stop=True)
        nc.vector.tensor_scalar(tmp[:],psb[:,:],gm_sb[:],None,AL.mult)
        nc.vector.tensor_tensor(acc[:],acc[:],tmp[:],AL.add)
        ot=pp.tile([128,512],F32,tag='ot')
        nc.vector.tensor_scalar(ot[:],po[:,:],1.0/N,v_sb[:],AL.mult,AL.add)
        fin=pp.tile([128,512],F32,tag='fin')
        nc.vector.tensor_tensor(fin[:],acc[:],ot[:],AL.add)
        dma(out_ext.ap(),fin[:])
    nc.compile()
    return nc
